# revision 2
# baseline (speedup 1.0000x reference)
"""Linear-chain CRF partition function (log Z) on 8 Trainium2 NeuronCores, v2.

Exp-domain recurrence p' = (ET^T p) * f as 128x128xN PE matmuls + elementwise
multiplies, with the multiply work spread across THREE engine paths per core:

  - 3 DVE streams (2 fused chains each, [128,512] tiles): DVE tensor_mul
    reads the f32 PSUM matmul output directly, fp8 features from SBUF.
  - 3 Pool streams ([128,512]): ACT copies PSUM->SBUF bf16, GPSIMD
    multiplies all-SBUF (it cannot read PSUM).
  - 2 log-domain half-streams ([128,256]): ACT Exp -> PE matmul -> ACT Ln ->
    PE identity-matmul adds ln(q) + raw feats into a PSUM group.  The
    feature "multiply" costs PE time (cheap) instead of DVE/Pool.

Per-slot latency (state RAW chain mul -> matmul -> mul, ~1.5-3us) far
exceeds per-op busy time, so each engine path runs ~3 independent streams.
The 1024-step scan splits into 112 chains (14/core); steps 0..13 are folded
on the host in f64 (fixes coverage parity and the exact-init special case),
every other chain runs 2 redundant warmup steps from a uniform state (the
positive transition matrix contracts direction error ~50x/step).  Chain
scales stitch via colsum ratios computed as transposed PE sums (state as
stationary, [ones|w_end] moving) accumulated in one PSUM tile; logs happen
on the host.  Features are fp8e4m3 exp(feat-3) for mul paths, raw bf16 for
the log path, all SBUF-resident; the modeled DMA pool is serial so chunks
are few, large, and issued in need-order on the SP queue.
"""

import numpy as np
import ml_dtypes

import concourse.bacc as bacc
import concourse.bass as bass
import concourse.tile as tile
from concourse import mybir
from concourse._compat import with_exitstack
from concourse.bass_utils import run_bass_kernel_spmd

B, S, T2 = 256, 1024, 128
NCORES = 8
SHIFT = 3.0           # exp-domain chains use f = exp(feat - SHIFT)
BX = -16.0 / 3.0      # log-domain chains: per-slot exp bias
W = 1                 # warmup slots per chain (direction contracts ~50x/step)
KHOST = 15            # steps 0..KHOST-1 folded on host (exact, f64)
ND, NP, NX = 3, 3, 2
L_D, G_D = 14, 2
L_P, G_P = 7, 2
L_X, G_X = 4, 2
STREAMS = ([("D", G_D, L_D)] * ND + [("P", G_P, L_P)] * NP
           + [("X", G_X, L_X)] * NX)
CPC = sum(g for _, g, _ in STREAMS)      # 14 chains per core
NCH = NCORES * CPC
BF16, F32, FP8 = mybir.dt.bfloat16, mybir.dt.float32, mybir.dt.float8e4
NPBF = ml_dtypes.bfloat16
NPF8 = ml_dtypes.float8_e4m3
AF = mybir.ActivationFunctionType

# consts blob layout (bf16, [T2, CT_W]):
CT_ET = 0             # [128] exp(trans).T
CT_GE = 128           # [2]  col0=ones col1=exp(trans[END])
CT_ID = 130           # [128] identity
CT_PI = 258           # [768]: [0:512] stream D0 init (core0 blk0 exact),
                      #        [256:768] all-ones for everything else
CT_W = CT_PI + 768

BLK0 = np.cumsum([0] + [g for _, g, _ in STREAMS])[:-1]


def _plan():
    starts = []
    t = KHOST + W  # chain0's warmup slots cover steps KHOST..KHOST+1 exactly
    for _ in range(NCORES):
        for kind, g, l in STREAMS:
            for _ in range(g):
                starts.append(t)
                t += l - W
    assert t == S, t
    return starts


STARTS = _plan()


@with_exitstack
def _body(ctx, tc, OUT_d, CT_d, F_d):
    nc = tc.nc
    const = ctx.enter_context(tc.tile_pool(name="const", bufs=1))
    fpool = ctx.enter_context(tc.tile_pool(name="f", bufs=1))
    spool = ctx.enter_context(tc.tile_pool(name="s", bufs=2))
    qspool = ctx.enter_context(tc.tile_pool(name="qs", bufs=2))
    xpool = ctx.enter_context(tc.tile_pool(name="x", bufs=2))
    dq = [ctx.enter_context(
        tc.tile_pool(name=f"dq{i}", bufs=1, space=bass.MemorySpace.PSUM))
        for i in range(ND)]
    # P0/P1 share one q bank: their q is live only matmul->copy (~1.2us of
    # a ~2.9us slot cadence), and PSUM tiles each cost a full 2KB bank.
    pq01 = ctx.enter_context(
        tc.tile_pool(name="pq01", bufs=1, space=bass.MemorySpace.PSUM))
    pq2 = ctx.enter_context(
        tc.tile_pool(name="pq2", bufs=1, space=bass.MemorySpace.PSUM))
    pq = [(pq01, "qp01"), (pq01, "qp01"), (pq2, "q5")]
    xq = ctx.enter_context(
        tc.tile_pool(name="xq", bufs=1, space=bass.MemorySpace.PSUM))
    smpool = ctx.enter_context(
        tc.tile_pool(name="sm", bufs=1, space=bass.MemorySpace.PSUM))

    # One act table serves Copy+Ln+Exp; without this the table-load pass
    # thrashes 1.3us loads between per-func default tables.
    nc.scalar.add_instruction(
        mybir.InstLoadActFuncSet(
            name=nc.get_next_instruction_name(), ins=[], outs=[],
            act_func_set_id=6,  # natural_log_exp_and_others
        )
    )
    # PE preheat: the p-state ramp (3us to full clock) starts with the first
    # matmul; run garbage matmuls from t~0 while DMAs are still in flight.
    gz = const.tile([T2, 512], BF16, tag="gz")
    nc.vector.memset(gz[:], 0.0)
    bxt = const.tile([T2, 1], F32, tag="bx")
    nc.vector.memset(bxt[:], BX)
    qheat = pq01.tile([T2, G_P * B], F32, tag="qp01")
    for _ in range(7):
        nc.tensor.matmul(qheat[:], gz[:, 0:128], gz[:], start=True, stop=True)

    ct = const.tile([T2, CT_W], BF16, tag="consts")
    nc.sync.dma_start(ct[:], CT_d[:])
    et = ct[:, CT_ET : CT_ET + 128]
    ge = ct[:, CT_GE : CT_GE + 2]
    idm = ct[:, CT_ID : CT_ID + 128]

    # Features: all SBUF-resident.  The modeled DMA pool is SERIAL
    # (~360B/ns), so transfers are issued on SP in need-order: consts, a
    # 4-slot first chunk per stream, then the rests.
    fts = []
    for i, (kind, g, l) in enumerate(STREAMS):
        ft = fpool.tile([T2, l, g * B], FP8, tag=f"ft{i}")
        fts.append(ft)
    CF = 4
    # need-order on the serial DMA pool: D firsts, X (whole, tiny), P
    # firsts, D mids, P rests, D tails
    for i in (0, 3, 4, 5, 1, 2):   # D0, P firsts, D1, D2
        nc.sync.dma_start(fts[i][:, 0:CF, :], F_d[i][:, 0:CF, :])
    for i in (6, 7):
        nc.sync.dma_start(fts[i][:], F_d[i][:])
    for i in (0, 1, 2):
        nc.sync.dma_start(fts[i][:, CF:9, :], F_d[i][:, CF:9, :])
    for i in (3, 4, 5):
        nc.sync.dma_start(fts[i][:, CF:L_P, :], F_d[i][:, CF:L_P, :])
    for i in (0, 1, 2):
        nc.sync.dma_start(fts[i][:, 9:L_D, :], F_d[i][:, 9:L_D, :])

    # initial states (ones everywhere; core0 D0 block0 = exact host state)
    st = [ct[:, CT_PI : CT_PI + 512] if i == 0
          else ct[:, CT_PI + 256 : CT_PI + 256 + g * B]
          for i, (kind, g, l) in enumerate(STREAMS)]
    xalpha = [None] * len(STREAMS)

    # transposed sums: per (chain, delta/end) event 4 columns of one PSUM
    # tile: [colsum_h0, wy_h0, colsum_h1, wy_h1], batch = partition row.
    smt = smpool.tile([T2, 8 * CPC], F32, tag="smt")

    def sums(i, state, g, ev):
        for gg in range(g):
            e4 = (2 * (BLK0[i] + gg) + ev) * 4
            for h in range(2):
                nc.tensor.matmul(
                    smt[:, e4 + 2 * h : e4 + 2 * h + 2],
                    state[:, gg * B + h * T2 : gg * B + (h + 1) * T2],
                    ge[:], start=True, stop=True)

    maxL = max(l for _, _, l in STREAMS)
    # interleave kinds in emission so same-kind streams don't phase-lock
    # into convoys (PE completes all D matmuls together -> DVE serializes)
    emit_order = [3, 0, 6, 4, 1, 7, 5, 2]
    # scheduling-only phase hints: stagger same-kind streams so their PE
    # matmuls don't complete in lockstep
    cad = {"D": 1.9e-3, "P": 2.7e-3, "X": 2.5e-3}
    phase = {0: 3.5e-3, 1: 4.1e-3, 2: 4.7e-3, 3: 4.3e-3, 4: 5.2e-3,
             5: 6.1e-3, 6: 5.0e-3, 7: 6.2e-3}
    for s in range(maxL + 1):
        for i in emit_order:
            kind, g, l = STREAMS[i]
            tc.tile_set_cur_wait(phase[i] + s * cad[kind])
            if s > l:
                continue
            wd = g * B
            if kind == "X" and s > 0:  # exp of current state (incl s == l)
                e = xpool.tile([T2, wd], BF16, tag=f"e{i}")
                nc.scalar.activation(e[:], xalpha[i][:], AF.Exp,
                                     bias=bxt[:], scale=1.0)
                st[i] = e
            if s == W:
                sums(i, st[i], g, 0)
            if s == l:
                sums(i, st[i], g, 1)
                continue
            if kind == "D":
                q = dq[i].tile([T2, wd], F32, tag=f"q{i}")
                nc.tensor.matmul(q[:], et[:], st[i][:], start=True, stop=True)
                sn = spool.tile([T2, wd], BF16, tag=f"s{i}")
                nc.vector.tensor_mul(sn[:], q[:], fts[i][:, s, :])
                st[i] = sn
            elif kind == "P":
                pool_, tag_ = pq[i - ND]
                q = pool_.tile([T2, wd], F32, tag=tag_)
                nc.tensor.matmul(q[:], et[:], st[i][:], start=True, stop=True)
                qs = qspool.tile([T2, wd], BF16, tag=f"qs{i}")
                nc.scalar.copy(qs[:], q[:])
                sn = spool.tile([T2, wd], BF16, tag=f"s{i}")
                nc.gpsimd.tensor_mul(sn[:], qs[:], fts[i][:, s, :])
                st[i] = sn
            else:  # X: q = M e; lnq; alpha' = I.lnq + I.feat (PSUM group)
                q = xq.tile([T2, wd], F32, tag=f"xq{i}")
                nc.tensor.matmul(q[:], et[:], st[i][:], start=True, stop=True)
                lnq = xpool.tile([T2, wd], BF16, tag=f"lnq{i}")
                nc.scalar.activation(lnq[:], q[:], AF.Ln)
                xa = xq.tile([T2, wd], F32, tag=f"xq{i}")
                xalpha[i] = xa
                nc.tensor.matmul(xa[:], idm[:], lnq[:],
                                 start=True, stop=False)
                nc.tensor.matmul(xa[:], idm[:], fts[i][:, s, :],
                                 start=False, stop=True)

    stage = const.tile([T2, 8 * CPC], F32, tag="stage")
    nc.vector.tensor_copy(stage[:], smt[:])
    nc.sync.dma_start(OUT_d[:], stage[:])


_NC_CACHE = {}


def _get_nc():
    if "nc" not in _NC_CACHE:
        nc = bacc.Bacc("TRN2", target_bir_lowering=False, debug=False)
        CT_d = nc.dram_tensor("CT", [T2, CT_W], BF16, kind="ExternalInput")
        F_d = [
            nc.dram_tensor(f"F{i}", [T2, l, g * B], FP8, kind="ExternalInput")
            for i, (kind, g, l) in enumerate(STREAMS)
        ]
        OUT_d = nc.dram_tensor("OUT", [T2, 8 * CPC], F32, kind="ExternalOutput")
        with tile.TileContext(nc) as tc:
            _body(tc, OUT_d, CT_d, F_d)
        nc.compile()
        _NC_CACHE["nc"] = nc
    return _NC_CACHE["nc"]


def _host_fold(feats, trans):
    """Exact f64 log-domain forward for steps 0..KHOST-1.
    Returns (init_bf16 [T2,B] = exp(alpha-m), m [B])."""
    E = np.exp(trans.astype(np.float64))
    alpha = np.full((B, T2), -1e5, np.float64)
    alpha[:, -1] = 0.0
    for t in range(KHOST):
        mm = alpha.max(-1, keepdims=True)
        alpha = mm + np.log(np.exp(alpha - mm) @ E.T) + feats[:, t, :]
    m = alpha.max(-1)
    init = np.exp(alpha - m[:, None]).T  # [T2, B]
    return init.astype(NPBF), m


def prepare_in_maps(feats, trans):
    feats = np.asarray(feats, dtype=np.float32)
    trans = np.asarray(trans, dtype=np.float32)
    assert feats.shape == (B, S, T2) and trans.shape == (T2, T2)

    with np.errstate(under="ignore", over="ignore"):
        # floor: blocked transitions give q=0 -> Ln=-inf -> 0*inf=NaN in the
        # identity matmul on the log path; 1e-30 keeps everything finite and
        # contributes ~e^-67 to colsums (negligible)
        ET = np.maximum(np.exp(trans).T, 1e-30)   # [from, to]
        F8 = np.exp(feats.transpose(2, 1, 0) - SHIFT).astype(NPF8)  # [T2,S,B]
    FXf = feats.transpose(2, 1, 0).astype(NPF8)   # raw (log domain), fp8
    init, m0 = _host_fold(feats, trans)

    CT = np.zeros((T2, CT_W), np.float32)
    CT[:, CT_ET : CT_ET + 128] = ET
    CT[:, CT_GE] = 1.0
    CT[:, CT_GE + 1] = np.exp(trans[-2, :])
    CT[:, CT_ID : CT_ID + 128] = np.eye(T2)
    CT[:, CT_PI : CT_PI + 768] = 1.0
    CT = np.repeat(CT[None], NCORES, 0).astype(NPBF)
    CT[0, :, CT_PI : CT_PI + B] = init

    in_maps = []
    ci = 0
    for k in range(NCORES):
        mp = {"CT": CT[k]}
        for i, (kind, g, l) in enumerate(STREAMS):
            t0s = [STARTS[ci + gg] - W for gg in range(g)]
            src = FXf if kind == "X" else F8
            blk = np.stack([src[:, t0 : t0 + l, :] for t0 in t0s], axis=2)
            mp[f"F{i}"] = np.ascontiguousarray(blk.reshape(T2, l, g * B))
            ci += g
        in_maps.append(mp)
    _NC_CACHE["m0"] = m0
    return in_maps


def postprocess(results):
    m0 = _NC_CACHE["m0"]
    logZ = m0.astype(np.float64).copy()
    ci = 0
    with np.errstate(divide="ignore"):
        for k, r in enumerate(results):
            out = r["OUT"].astype(np.float64)

            def col(e, j):  # j: 0=colsum 1=w.y -> [B]
                return np.concatenate([out[:, 4 * e + j], out[:, 4 * e + 2 + j]])

            li = 0
            for kind, g, l in STREAMS:
                for gg in range(g):
                    last = ci == NCH - 1
                    logZ += np.log(col(2 * li + 1, 1 if last else 0))
                    if ci > 0:
                        logZ -= np.log(col(2 * li, 0))
                        nsl = l - W
                    else:
                        nsl = l  # chain 0: warmup slots are real
                    logZ += (-BX if kind == "X" else SHIFT) * nsl
                    ci += 1
                    li += 1
    return logZ.astype(np.float32)


def run(feats, trans, trace=False, **spmd_kwargs):
    nc = _get_nc()
    in_maps = prepare_in_maps(feats, trans)
    res = run_bass_kernel_spmd(
        nc, in_maps, list(range(NCORES)), trace=trace, **spmd_kwargs
    )
    return postprocess(res.results), res


def kernel(feats, trans):
    out, _ = run(feats, trans, trace=False)
    return out
